# revision 2
# baseline (speedup 1.0000x reference)
"""LIF neuron with soft reset — Trainium2 Bass kernel, 8-way data parallel.

Problem: x (T=32, B=16, C=128, H=32, W=32) f32. Sequential scan over T:
    mem = 0.25*mem + x[t];  s[t] = (mem >= 1);  mem -= s[t]
Returns spikes (same shape, f32 values in {0,1}).

Sharding: batch dim B=16 split 2-per-core across 8 NeuronCores; the scan is
elementwise over (B,C,H,W) so cores are fully independent. Per-core slice of
one timestep = 2*128*32*32 = 262144 contiguous floats -> [128 x 2048] f32.

Scaled-threshold formulation: carry PRE-reset state P_t = 4^t * mem_pre_t,
with y_t = 4^t * x_t pre-scaled on the host (exact power-of-two multiply).
    P_t   = (P_{t-1} - 4^{t-1} * [P_{t-1} >= 4^{t-1}]) + y_t
    s_t   = [P_t >= 4^t]
Power-of-two rescaling commutes with IEEE-754 rounding; the reset subtract
is exact; the only rounding per step is the +y_t add -- so every P_t is
bit-identical to the reference fp32 sequence.

v6 engine assignment (all four compute engines + DMA, every stage exact):
    DVE    : custom fused op LIF_STEP_ANT per step (the serial recurrence),
             out = (Src0 - C0*(Src0 > C1)) + Src1, over 6 rotating state
             tiles (deep rotation decouples the WAR handshake with ACT).
    ACT    : Sign(P - prevfloat(4^t)) -> u8 spike in {0,1} (fp32->u8
             conversion saturates -1 to 0; equality -> 0, matching the
             strict-> / >= boundary exactly).  Also copies the packed PSUM
             group out with scale=512.
    PE     : packs 8 steps/byte: spike u8 tile bitcast to fp8e4 (0x01 =
             2^-9 denormal), matmul with diag(2^k) fp8 stationary,
             accumulated in PSUM fp32 (exact: 2^-9 * 2^k * 512 = 2^k).
    DMA    : x loaded 4 timesteps per transfer (4 MB) through 4 rotating
             buffers (~425 GB/s); output is bit-packed (T/8 = 4 u8 maps,
             32x less write traffic than f32); host unpacks bits.
"""

import numpy as np

T, B, C, H, W = 32, 16, 128, 32, 32
NCORES = 8
BPC = B // NCORES          # batches per core
P = 128                    # SBUF partitions
F = (BPC * C * H * W) // P # 2048 free-dim elements per step
LB = 4                     # timesteps per x load batch
NSTATE = 6                 # rotating DVE state tiles
G8 = T // 8                # packed output groups

_cache = {}


def _prevfloat(v: float) -> float:
    return float(np.nextafter(np.float32(v), np.float32(0)))


def _register_lif_op():
    """Append the fused LIF step op to the custom-DVE registry (idempotent).

    out = (Src0 - s0 * (Src0 > s1)) + Src1
    s0 = 4^(t-1) reset amount, s1 = prevfloat(4^(t-1)) so the strict > equals
    the reference's >= on fp32 values.
    """
    from concourse import dve_ops
    from concourse.dve_spec import Spec, Src0, Src1, C0, C1, lower, _has_src1
    from concourse.dve_uop import DveOpSpec

    for op in dve_ops.OPS:
        if op.name == "LIF_STEP_ANT":
            return op

    spec = Spec(
        body=(Src0 - C0 * (Src0 > C1)) + Src1,
        reference=lambda in0, in1, s0, s1, imm2: (
            in0 - s0 * (in0 > s1).astype(np.float32)
        )
        + in1,
    )
    op = dve_ops.DveOp("LIF_STEP_ANT", spec, subdim=False, uops_sha={})
    dve_ops.OPS.append(op)
    dve_ops.CUSTOM_DVE_SPECS[op.name] = op.spec
    dve_ops._SUB_OPCODE_FOR_NAME[op.name] = (
        dve_ops._CUSTOM_DVE_ROW_BASE + len(dve_ops.OPS) - 1
    )
    # self-pin the sha exactly the way DveOp.compile() derives it
    for ver in ("v3", "v4"):
        try:
            compiled = DveOpSpec(
                name=op.name,
                opcode=dve_ops.get_dve_sub_opcode(op.name),
                uops=lower(spec, ver=ver),
                rd1_en=_has_src1(spec),
            )
            op.uops_sha[ver] = compiled.sha(ver)
        except Exception:
            pass
    return op


def _build(reps: int = 1):
    import concourse.bacc as bacc
    import concourse.mybir as mybir
    from concourse.bass import MemorySpace
    from concourse.tile import TileContext

    nc = bacc.Bacc(None, target_bir_lowering=False)
    x_d = nc.dram_tensor("x", [T, P, F], mybir.dt.float32, kind="ExternalInput")
    w_d = nc.dram_tensor("w", [P, 8 * 128], mybir.dt.float8e4, kind="ExternalInput")
    th_d = nc.dram_tensor("th", [P, T], mybir.dt.float32, kind="ExternalInput")
    o_d = nc.dram_tensor("o", [G8, P, F], mybir.dt.uint8, kind="ExternalOutput")

    fp32 = mybir.dt.float32
    u8 = mybir.dt.uint8
    fp8 = mybir.dt.float8e4
    Act = mybir.ActivationFunctionType
    lif = _register_lif_op()

    with TileContext(nc) as tc:
        with (
            tc.tile_pool(name="mem", bufs=1) as mempool,
            tc.tile_pool(name="xin", bufs=4) as xpool,
            tc.tile_pool(name="spk", bufs=6) as spool,
            tc.tile_pool(name="pk", bufs=2) as pkpool,
            tc.tile_pool(name="wp", bufs=1) as wpool,
            tc.tile_pool(name="ps", bufs=2, space=MemorySpace.PSUM) as pspool,
        ):
            wt = wpool.tile([P, 8 * 128], fp8, name="w", tag="w")
            nc.sync.dma_start(out=wt, in_=w_d[:, :])
            th = wpool.tile([P, T], fp32, name="th", tag="th")
            nc.sync.dma_start(out=th, in_=th_d[:, :])
            pst = [
                mempool.tile([P, F], fp32, name=f"p{i}", tag=f"p{i}")
                for i in range(NSTATE)
            ]
            for _ in range(reps):  # reps>1 only for benchmarking
                nc.vector.memset(pst[0], 0.0)
                for g in range(G8):
                    acc = pspool.tile([P, F], fp32, name="acc", tag="acc")
                    for k in range(8):
                        t = 8 * g + k
                        if t % LB == 0:
                            xt4 = xpool.tile([P, LB * F], fp32, name="x", tag="x")
                            nc.sync.dma_start(
                                out=xt4.rearrange("p (t f) -> p t f", t=LB),
                                in_=x_d[t : t + LB].rearrange("t p f -> p t f"),
                            )
                        # reset uses the PREVIOUS step's threshold; t=0 resets
                        # nothing (state is 0), s0=0 makes the op a plain add
                        rst = float(4.0 ** (t - 1)) if t > 0 else 0.0
                        rthr = _prevfloat(4.0 ** (t - 1)) if t > 0 else 1.0
                        src = pst[t % NSTATE]
                        dst = pst[(t + 1) % NSTATE]
                        yt = xt4[:, (t % LB) * F : (t % LB + 1) * F]
                        nc.vector._custom_dve(
                            lif, out=dst, in0=src, in1=yt, s0=rst, s1=rthr
                        )
                        st = spool.tile([P, F], u8, name="s", tag="s")
                        nc.scalar.activation(
                            st, dst, Act.Sign, bias=th[:, t : t + 1]
                        )
                        s8 = st.bitcast(fp8)
                        for c0 in range(0, F, 512):
                            nc.tensor.matmul(
                                acc[:, c0 : c0 + 512],
                                wt[:, 128 * k : 128 * (k + 1)],
                                s8[:, c0 : c0 + 512],
                                start=(k == 0),
                                stop=(k == 7),
                            )
                    pk = pkpool.tile([P, F], u8, name="pk", tag="pk")
                    nc.scalar.activation(pk, acc, Act.Copy, scale=512.0)
                    nc.sync.dma_start(out=o_d[g], in_=pk)
    nc.finalize()
    return nc


def _consts():
    import ml_dtypes

    w = np.zeros((P, 8 * 128), ml_dtypes.float8_e4m3)
    eye = np.eye(128)
    for k in range(8):
        w[:, 128 * k : 128 * (k + 1)] = (2.0**k * eye).astype(ml_dtypes.float8_e4m3)
    th = np.empty((P, T), np.float32)
    for t in range(T):
        th[:, t] = -np.float32(_prevfloat(4.0**t))
    return w, th


def kernel(x: np.ndarray) -> np.ndarray:
    from concourse.bass_utils import run_bass_kernel_spmd

    assert x.shape == (T, B, C, H, W) and x.dtype == np.float32
    if "nc" not in _cache:
        _cache["nc"] = _build()
    nc = _cache["nc"]

    # host-side pre-scale: y_t = 4^t * x_t (exact power-of-two multiply)
    scale = (4.0 ** np.arange(T, dtype=np.float64)).astype(np.float32)
    y = x * scale[:, None, None, None, None]
    w, th = _consts()

    in_maps = []
    for c in range(NCORES):
        yk = np.ascontiguousarray(y[:, c * BPC : (c + 1) * BPC]).reshape(T, P, F)
        in_maps.append({"x": yk, "w": w, "th": th})

    res = run_bass_kernel_spmd(nc, in_maps, core_ids=list(range(NCORES)))
    _cache["last_result"] = res

    # unpack: byte g bit k (LSB-first) = spike at t = 8g+k
    bits = np.arange(8, dtype=np.uint8)
    out = np.empty((T, B, C, H, W), dtype=np.float32)
    for c in range(NCORES):
        pk = res.results[c]["o"]  # [G8, P, F] u8
        sp = (pk[:, None] >> bits[None, :, None, None]) & np.uint8(1)
        out[:, c * BPC : (c + 1) * BPC] = (
            sp.reshape(T, BPC, C, H, W).astype(np.float32)
        )
    return out
